# revision 36
# baseline (speedup 1.0000x reference)
"""GaussianImage (Cholesky) renderer on 8 trn2 NeuronCores.

Tile-parallel over the pixel grid: the 256x256 image is cut into 16x16-pixel
tiles (256/frame, 512 total for T=2).  The host bins gaussians to tiles
(bbox intersect via a conservative support radius; outside it exp(-sigma)
is negligible), computes per-(tile,gaussian) quadratic-form coefficients in
f64, and bin-packs the (tile,gaussian) pairs onto the 128 SBUF/PSUM
partitions: each tile occupies only as many rows as it has gaussians, so a
core's whole workload fits in ~4 partition-groups of 128 rows.

Device-side math per core (all heavy compute on PE/ACT/DVE engines):

  sigma = coefT(24,128)^T @ basis(24,512)     [TensorE bf16, hi/lo split]
  alpha = Exp(-sigma)                         [ScalarE, PSUM->SBUF bf16]
  img   = wBlock(128,CH)^T @ alpha(128,256)   [TensorE bf16]
  out   = clamp(img, 0, 1)                    [DVE, fused]

Precision: coefficients are split into bf16 hi + lo parts (~17 mantissa
bits) and the basis uses centered half-integer local coords, which are
EXACT in bf16 - so the sigma matmul carries full effective precision while
running at bf16 speed (1 cycle/col vs 4 for fp32).

K-stacking: 2 groups x 6 coefs x {hi,lo} = 24 contraction rows per
512-column matmul share one PE weight load and one block-diagonal basis;
every image-matmul column amortizes all tiles of a group via a
block-diagonal weight matrix.  Stage-2 matmul pairs land in one PSUM tile
through PE column-group placement so a single clamp covers two groups.

Each pixel is owned by exactly one tile -> no cross-core reduction; the
clamp happens on-device after the full per-pixel sum.
"""

import os
import numpy as np
import ml_dtypes

T, N, H, W = 2, 512, 256, 256
TILE = 16
NT = H // TILE            # 16 tiles per axis
N_TILES = NT * NT         # 256 per frame
N_ENTRIES = T * N_TILES   # 512
N_CORES = 8
PIX = TILE * TILE         # 256
NE_CAP = 21               # entries per group (3*NE_CAP = 63 output rows)
SIGMA_CUT = 10.0          # exp(-10) ~ 4.5e-5: negligible vs 2e-2 gate
DROP_MIN_SIGMA = 9.0      # overflow slots may be dropped above this

_CACHE = {}

BF16 = ml_dtypes.bfloat16


def _build_nc(G, CHMAX):
    """G groups of <=128 (tile,gaussian) rows; CHMAX output rows/group."""
    import concourse.bass as bass
    import concourse.mybir as mybir
    from concourse.tile import TileContext
    import bass_rust

    NCB = -(-G // 2)          # chunks: 2 groups per 512-col PSUM bank
    CT = NCB * 128            # coef columns in the combined input

    f32 = mybir.dt.float32
    bf16 = mybir.dt.bfloat16
    Alu = mybir.AluOpType
    Act = mybir.ActivationFunctionType

    half = (CT + 512) // 2
    nc = bass.Bass("TRN2")
    # stored column-folded as [48, half] so the DMA spreads over 48 rows
    cb_d = nc.dram_tensor("cb", [48, half], bf16, kind="ExternalInput")
    wb_d = nc.dram_tensor("wblk", [128, G * CHMAX], bf16, kind="ExternalInput")
    out_d = nc.dram_tensor("out", [2 * CHMAX, NCB * PIX], bf16,
                           kind="ExternalOutput")

    with TileContext(nc) as tc:
        with tc.tile_pool(name="const", bufs=1) as cpool, \
             tc.tile_pool(name="ps_sig", bufs=2, space="PSUM") as ps_sig, \
             tc.tile_pool(name="ps_img", bufs=2, space="PSUM") as ps_img:

            # sync carries the coefs + shared 2-group basis (unblocks all
            # stage-1 matmuls); scalar carries the image weights (needed
            # ~2us later by stage 2)
            cba = cpool.tile([24, CT + 512], bf16, tag="cba")
            wb = cpool.tile([128, G * CHMAX], bf16, tag="wb")
            alpha = cpool.tile([128, G * PIX], bf16, tag="alpha")
            st = cpool.tile([2 * CHMAX, NCB * PIX], bf16, tag="st")

            nc.sync.dma_start(
                out=cba[:].rearrange("r (h c) -> r h c", h=2),
                in_=cb_d[:].rearrange("(h r) c -> r h c", h=2))
            nc.scalar.dma_start(out=wb, in_=wb_d[:])

            # warm the Exp ACT table while input DMAs are in flight
            warm = cpool.tile([1, 1], f32, tag="warm")
            nc.vector.memset(warm, 0.0)
            nc.scalar.activation(warm, warm, Act.Exp)

            # stage 1: sigma = coefT^T @ basis; each 512-col chunk covers 2
            # groups via K=24 (6 coefs x hi/lo x 2), sharing one basis block
            for k in range(NCB):
                gk = min(2, G - 2 * k)
                K = 12 * gk
                nc_cols = gk * PIX
                sig = ps_sig.tile([128, nc_cols], f32, tag="sig",
                                  name=f"sig{k}")
                nc.tensor.matmul(
                    sig, cba[0:K, k * 128:(k + 1) * 128],
                    cba[0:K, CT:CT + nc_cols],
                    start=True, stop=True)
                nc.scalar.activation(
                    alpha[:, 512 * k:512 * k + nc_cols], sig,
                    Act.Exp, scale=-1.0)

            # stage 2: per group, img = wBlock^T @ alpha, then clamp to [0,1].
            # Groups are paired into one [2*CHMAX, 256] PSUM tile via PE
            # column-group placement so a single DVE clamp covers both.
            for cb2 in range(NCB):
                img = ps_img.tile([2 * CHMAX, PIX], f32, tag="img",
                                  name=f"img{cb2}")
                npair = min(2, G - 2 * cb2)
                for h in range(npair):
                    g = 2 * cb2 + h
                    nc.tensor.matmul(
                        img[h * CHMAX:(h + 1) * CHMAX, :],
                        wb[:, g * CHMAX:(g + 1) * CHMAX],
                        alpha[:, g * PIX:(g + 1) * PIX],
                        start=True, stop=True,
                        tile_position=(0, h * CHMAX))
                rows = npair * CHMAX
                nc.vector.tensor_scalar(
                    out=st[0:rows, cb2 * PIX:(cb2 + 1) * PIX],
                    in0=img[0:rows, :],
                    scalar1=0.0, scalar2=1.0, op0=Alu.max, op1=Alu.min)

            halfA = (NCB // 2) * PIX
            if halfA:
                nc.sync.dma_start(out=out_d[:, 0:halfA], in_=st[:, 0:halfA])
            # split the tail block by band so the final DMA only carries the
            # last-clamped group
            nc.scalar.dma_start(out=out_d[0:CHMAX, halfA:],
                                in_=st[0:CHMAX, halfA:])
            nc.sync.dma_start(out=out_d[CHMAX:, halfA:],
                              in_=st[CHMAX:, halfA:])

    bass_rust.generate_event_semaphores(nc)
    return nc


def _host_params(xyz, cholesky, opacity, features_dc):
    """f64 host math mirroring the reference's per-gaussian transforms."""
    means = np.tanh(xyz.astype(np.float64))
    cx = 0.5 * W * (means[..., 0] + 1.0)               # (T,N)
    cy = 0.5 * H * (means[..., 1] + 1.0)
    chol = cholesky.astype(np.float64) + np.array([0.5, 0.0, 0.5])
    l0, l1, l2 = chol[..., 0], chol[..., 1], chol[..., 2]
    sxx, sxy, syy = l0 * l0, l0 * l1, l1 * l1 + l2 * l2
    det = sxx * syy - sxy * sxy
    ca, cb_, cc = syy / det, -sxy / det, sxx / det     # conic
    tr = sxx + syy
    lam = tr / 2 + np.sqrt(np.maximum(tr * tr / 4 - det, 0.0))
    opac = 1.0 / (1.0 + np.exp(-opacity.astype(np.float64)[:, 0]))   # (N,)
    colors = 1.0 / (1.0 + np.exp(-features_dc.astype(np.float64)))   # (N,3)
    wcol = colors * opac[:, None]
    return cx, cy, ca, cb_, cc, lam, wcol


def _bin_entries(cx, cy, lam, S):
    """Route gaussians to 16x16 tiles; drop only provably-invisible overflow."""
    r = np.sqrt(2.0 * SIGMA_CUT * np.maximum(lam, 0.0)) + 1.0
    entries = []
    for t in range(T):
        x0 = np.clip(((cx[t] - r[t]) // TILE).astype(int), 0, NT - 1)
        x1 = np.clip(((cx[t] + r[t]) // TILE).astype(int), 0, NT - 1)
        y0 = np.clip(((cy[t] - r[t]) // TILE).astype(int), 0, NT - 1)
        y1 = np.clip(((cy[t] + r[t]) // TILE).astype(int), 0, NT - 1)
        buckets = [[[] for _ in range(NT)] for _ in range(NT)]
        for n in range(N):
            for ty in range(y0[n], y1[n] + 1):
                for tx in range(x0[n], x1[n] + 1):
                    buckets[ty][tx].append(n)
        for ty in range(NT):
            for tx in range(NT):
                idxs = buckets[ty][tx]
                if len(idxs) > S:
                    ms = []
                    for n in idxs:
                        ddx = max(0.0, tx * TILE - cx[t, n],
                                  cx[t, n] - (tx * TILE + TILE - 1))
                        ddy = max(0.0, ty * TILE - cy[t, n],
                                  cy[t, n] - (ty * TILE + TILE - 1))
                        ms.append((ddx * ddx + ddy * ddy) / (2.0 * lam[t, n]))
                    order = np.argsort(ms)
                    assert ms[order[S]] >= DROP_MIN_SIGMA, \
                        "tile overflow with visible gaussians"
                    idxs = [idxs[i] for i in order[:S]]
                entries.append((t, ty, tx, idxs))
    return entries


def _pack(entries):
    """Global first-fit-decreasing pack of tiles onto 128-row groups."""
    sizes = np.array([len(e[3]) for e in entries])
    order = np.argsort(-sizes, kind="stable")
    bins = []            # list of [rows_used, [entry_idx, ...]]
    for i in order:
        s = int(sizes[i])
        if s == 0:
            continue
        for b in bins:
            if b[0] + s <= 128 and len(b[1]) < NE_CAP:
                b[0] += s
                b[1].append(int(i))
                break
        else:
            bins.append([s, [int(i)]])
    # deal bins (largest first) round-robin to cores
    core_bins = [[] for _ in range(N_CORES)]
    for j, b in enumerate(sorted(bins, key=lambda b: -b[0])):
        core_bins[j % N_CORES].append(b[1])
    G = max(len(cb) for cb in core_bins)
    # round up to a multiple of 32: engine APs need 32-aligned partition
    # bases; <=64 keeps two groups PE-placeable in one PSUM tile
    CHMAX = -(-3 * max(len(b[1]) for b in bins) // 32) * 32
    assert CHMAX <= 64, "NE_CAP too large for paired stage-2 placement"
    return core_bins, G, CHMAX


def _pack_core(bins, G, CHMAX, entries, cx, cy, ca, cb_, cc, wcol):
    """Build one core's coef ('cb' head), weight-block, and placement meta."""
    NCB = -(-G // 2)
    ct = np.zeros((24, NCB * 128), BF16)
    wb = np.zeros((128, G * CHMAX), BF16)
    meta = []
    for g, ent_list in enumerate(bins):
        b, gi = divmod(g, 2)
        r = 0
        for j, ei in enumerate(ent_list):
            t, ty, tx, idxs = entries[ei]
            n = np.asarray(idxs)
            k = len(n)
            ex = cx[t, n] - (tx * TILE + (TILE - 1) / 2.0)
            ey = cy[t, n] - (ty * TILE + (TILE - 1) / 2.0)
            A, Bc, Cc = ca[t, n], cb_[t, n], cc[t, n]
            C = np.empty((6, k), np.float64)
            C[0] = 0.5 * A
            C[1] = Bc
            C[2] = 0.5 * Cc
            C[3] = -(A * ex + Bc * ey)
            C[4] = -(Cc * ey + Bc * ex)
            C[5] = 0.5 * A * ex * ex + 0.5 * Cc * ey * ey + Bc * ex * ey
            hi = C.astype(BF16)
            lo = (C - hi.astype(np.float64)).astype(np.float32).astype(BF16)
            cols = slice(b * 128 + r, b * 128 + r + k)
            ct[12 * gi:12 * gi + 6, cols] = hi
            ct[12 * gi + 6:12 * gi + 12, cols] = lo
            wb[r:r + k, g * CHMAX + 3 * j:g * CHMAX + 3 * j + 3] = \
                wcol[n].astype(BF16)
            meta.append((g, j, ei))
            r += k
    return ct, wb, meta


def _basis_block():
    """[24, 2*256] bf16: centered half-integer monomials, exact in bf16."""
    x = np.arange(PIX, dtype=np.float64) % TILE - (TILE - 1) / 2.0
    y = np.arange(PIX, dtype=np.float64) // TILE - (TILE - 1) / 2.0
    rows = np.stack([x * x, x * y, y * y, x, y, np.ones(PIX)])  # (6,256)
    bt = np.zeros((24, 2 * PIX), np.float64)
    for gi in range(2):
        cols = slice(gi * PIX, (gi + 1) * PIX)
        bt[12 * gi:12 * gi + 6, cols] = rows
        bt[12 * gi + 6:12 * gi + 12, cols] = rows
    return bt.astype(BF16)


def _ensure_ntff_hook():
    """Provide antenv.axon_hooks (missing in this image) so trace=True works."""
    import sys, types, ctypes, contextlib
    if "antenv.axon_hooks" in sys.modules:
        return
    so_path = "/opt/axon/libaxon_pjrt.so"
    if not os.path.exists(so_path):
        return
    lib = ctypes.CDLL(so_path)
    if not hasattr(lib, "axon_start_nrt_profile"):
        return
    lib.axon_start_nrt_profile.argtypes = [ctypes.POINTER(ctypes.c_int64), ctypes.c_size_t]
    lib.axon_start_nrt_profile.restype = ctypes.c_int64
    lib.axon_stop_nrt_profile.argtypes = [ctypes.c_char_p]
    lib.axon_stop_nrt_profile.restype = ctypes.c_int64

    @contextlib.contextmanager
    def _hook(output_dir, device_ids):
        import jax
        jax.devices()
        if device_ids:
            ids = (ctypes.c_int64 * len(device_ids))(*device_ids)
            rc = lib.axon_start_nrt_profile(ids, len(device_ids))
        else:
            rc = lib.axon_start_nrt_profile(None, 0)
        if rc != 0:
            raise RuntimeError(f"axon_start_nrt_profile rc={rc}")
        try:
            yield
        finally:
            n = lib.axon_stop_nrt_profile(str(output_dir).encode())
            print(f"profile: {n} file(s) written to {output_dir}")

    mod = types.ModuleType("antenv.axon_hooks")
    mod.get_axon_ntff_profile_hook = lambda: _hook
    mod.set_axon_ntff_profile_hook = lambda h: None
    sys.modules["antenv.axon_hooks"] = mod


def kernel(xyz, cholesky, opacity, features_dc):
    from concourse import bass_utils

    xyz = np.asarray(xyz, np.float32)
    cholesky = np.asarray(cholesky, np.float32)
    opacity = np.asarray(opacity, np.float32)
    features_dc = np.asarray(features_dc, np.float32)

    cx, cy, ca, cb_, cc, lam, wcol = _host_params(
        xyz, cholesky, opacity, features_dc)

    entries = _bin_entries(cx, cy, lam, 128)
    core_bins, G, CHMAX = _pack(entries)

    if (G, CHMAX) not in _CACHE:
        _CACHE[(G, CHMAX)] = _build_nc(G, CHMAX)
    nc = _CACHE[(G, CHMAX)]

    bt = _basis_block()
    in_maps = []
    metas = []
    for c in range(N_CORES):
        ct, wb, meta = _pack_core(core_bins[c], G, CHMAX, entries,
                                  cx, cy, ca, cb_, cc, wcol)
        cbfull = np.concatenate([ct, bt], axis=1)
        half = cbfull.shape[1] // 2
        in_maps.append({"cb": np.concatenate(
                            [cbfull[:, 0:half], cbfull[:, half:]], axis=0),
                        "wblk": wb})
        metas.append(meta)

    trace = bool(int(os.environ.get("GS_TRACE", "0")))
    _ensure_ntff_hook()   # harmless if tracing is off; needed if forced on
    res = bass_utils.run_bass_kernel_spmd(
        nc, in_maps, core_ids=list(range(N_CORES)), trace=trace)
    kernel.last_result = res

    img = np.zeros((T, 3, H, W), np.float32)
    for c in range(N_CORES):
        o = res.results[c]["out"].astype(np.float32)
        for g, j, ei in metas[c]:
            t, ty, tx, _ = entries[ei]
            blk = o[(g % 2) * CHMAX + 3 * j:(g % 2) * CHMAX + 3 * j + 3,
                    (g // 2) * PIX:(g // 2 + 1) * PIX]
            img[t, :, ty * TILE:(ty + 1) * TILE,
                tx * TILE:(tx + 1) * TILE] = blk.reshape(3, TILE, TILE)
    return img


# revision 38
# speedup vs baseline: 1.0278x; 1.0278x over previous
"""GaussianImage (Cholesky) renderer on 8 trn2 NeuronCores.

Tile-parallel over the pixel grid: the 256x256 image is cut into 16x16-pixel
tiles (256/frame, 512 total for T=2).  The host bins gaussians to tiles
(bbox intersect via a conservative support radius; outside it exp(-sigma)
is negligible), computes per-(tile,gaussian) quadratic-form coefficients in
f64, and bin-packs the (tile,gaussian) pairs onto the 128 SBUF/PSUM
partitions: each tile occupies only as many rows as it has gaussians, so a
core's whole workload fits in ~4 partition-groups of 128 rows.

Device-side math per core (all heavy compute on PE/ACT/DVE engines):

  sigma = coefT(24,128)^T @ basis(24,512)     [TensorE bf16, hi/lo split]
  alpha = Exp(-sigma)                         [ScalarE, PSUM->SBUF bf16]
  img   = wBlock(128,CH)^T @ alpha(128,256)   [TensorE bf16]
  out   = clamp(img, 0, 1)                    [DVE, fused]

Precision: coefficients are split into bf16 hi + lo parts (~17 mantissa
bits) and the basis uses centered half-integer local coords, which are
EXACT in bf16 - so the sigma matmul carries full effective precision while
running at bf16 speed (1 cycle/col vs 4 for fp32).

K-stacking: 2 groups x 6 coefs x {hi,lo} = 24 contraction rows per
512-column matmul share one PE weight load and one block-diagonal basis;
every image-matmul column amortizes all tiles of a group via a
block-diagonal weight matrix.  Stage-2 matmul pairs land in one PSUM tile
through PE column-group placement so a single clamp covers two groups.

Each pixel is owned by exactly one tile -> no cross-core reduction; the
clamp happens on-device after the full per-pixel sum.
"""

import os
import numpy as np
import ml_dtypes

T, N, H, W = 2, 512, 256, 256
TILE = 16
NT = H // TILE            # 16 tiles per axis
N_TILES = NT * NT         # 256 per frame
N_ENTRIES = T * N_TILES   # 512
N_CORES = 8
PIX = TILE * TILE         # 256
NE_CAP = 21               # entries per group (3*NE_CAP = 63 output rows)
SIGMA_CUT = 10.0          # exp(-10) ~ 4.5e-5: negligible vs 2e-2 gate
DROP_MIN_SIGMA = 9.0      # overflow slots may be dropped above this

_CACHE = {}

BF16 = ml_dtypes.bfloat16


def _build_nc(G, CHMAX):
    """G groups of <=128 (tile,gaussian) rows; CHMAX output rows/group."""
    import concourse.bass as bass
    import concourse.mybir as mybir
    from concourse.tile import TileContext
    import bass_rust

    NCB = -(-G // 2)          # chunks: 2 groups per 512-col PSUM bank
    CT = NCB * 128            # coef columns in the combined input

    f32 = mybir.dt.float32
    bf16 = mybir.dt.bfloat16
    Alu = mybir.AluOpType
    Act = mybir.ActivationFunctionType

    half = (CT + 512) // 2
    nc = bass.Bass("TRN2")
    # stored column-folded as [48, half] so the DMA spreads over 48 rows
    cb_d = nc.dram_tensor("cb", [48, half], bf16, kind="ExternalInput")
    wb_d = nc.dram_tensor("wblk", [128, G * CHMAX], bf16, kind="ExternalInput")
    out_d = nc.dram_tensor("out", [2 * CHMAX, NCB * PIX], bf16,
                           kind="ExternalOutput")

    with TileContext(nc) as tc:
        with tc.tile_pool(name="const", bufs=1) as cpool, \
             tc.tile_pool(name="ps_sig", bufs=2, space="PSUM") as ps_sig, \
             tc.tile_pool(name="ps_img", bufs=2, space="PSUM") as ps_img:

            # sync carries the coefs + shared 2-group basis (unblocks all
            # stage-1 matmuls); scalar carries the image weights (needed
            # ~2us later by stage 2)
            cba = cpool.tile([24, CT + 512], bf16, tag="cba")
            wb = cpool.tile([128, G * CHMAX], bf16, tag="wb")
            alpha = cpool.tile([128, G * PIX], bf16, tag="alpha")
            st = cpool.tile([2 * CHMAX, NCB * PIX], bf16, tag="st")

            nc.sync.dma_start(
                out=cba[:].rearrange("r (h c) -> r h c", h=2),
                in_=cb_d[:].rearrange("(h r) c -> r h c", h=2))
            nc.scalar.dma_start(out=wb, in_=wb_d[:])

            # warm the Exp ACT table while input DMAs are in flight
            warm = cpool.tile([1, 1], f32, tag="warm")
            nc.vector.memset(warm, 0.0)
            nc.scalar.activation(warm, warm, Act.Exp)

            # stage 1: sigma = coefT^T @ basis; each 512-col chunk covers 2
            # groups via K=24 (6 coefs x hi/lo x 2), sharing one basis block
            for k in range(NCB):
                gk = min(2, G - 2 * k)
                K = 12 * gk
                nc_cols = gk * PIX
                sig = ps_sig.tile([128, nc_cols], f32, tag="sig",
                                  name=f"sig{k}")
                nc.tensor.matmul(
                    sig, cba[0:K, k * 128:(k + 1) * 128],
                    cba[0:K, CT:CT + nc_cols],
                    start=True, stop=True)
                nc.scalar.activation(
                    alpha[:, 512 * k:512 * k + nc_cols], sig,
                    Act.Exp, scale=-1.0)

            # stage 2: per group, img = wBlock^T @ alpha, then clamp to [0,1].
            # Groups are paired into one [2*CHMAX, 256] PSUM tile via PE
            # column-group placement so a single DVE clamp covers both.
            for cb2 in range(NCB):
                img = ps_img.tile([2 * CHMAX, PIX], f32, tag="img",
                                  name=f"img{cb2}")
                npair = min(2, G - 2 * cb2)
                for h in range(npair):
                    g = 2 * cb2 + h
                    nc.tensor.matmul(
                        img[h * CHMAX:(h + 1) * CHMAX, :],
                        wb[:, g * CHMAX:(g + 1) * CHMAX],
                        alpha[:, g * PIX:(g + 1) * PIX],
                        start=True, stop=True,
                        tile_position=(0, h * CHMAX))
                rows = npair * CHMAX
                nc.vector.tensor_scalar(
                    out=st[0:rows, cb2 * PIX:(cb2 + 1) * PIX],
                    in0=img[0:rows, :],
                    scalar1=0.0, scalar2=1.0, op0=Alu.max, op1=Alu.min)

            halfA = (NCB // 2) * PIX
            if halfA:
                nc.sync.dma_start(out=out_d[:, 0:halfA], in_=st[:, 0:halfA])
            # split the tail block by band (parallel queues); skip bands the
            # last column block doesn't populate
            last_rows = (G - 2 * (NCB - 1)) * CHMAX
            nc.scalar.dma_start(
                out=out_d[0:min(CHMAX, last_rows), halfA:],
                in_=st[0:min(CHMAX, last_rows), halfA:])
            if last_rows > CHMAX:
                nc.sync.dma_start(out=out_d[CHMAX:last_rows, halfA:],
                                  in_=st[CHMAX:last_rows, halfA:])

    bass_rust.generate_event_semaphores(nc)
    return nc


def _host_params(xyz, cholesky, opacity, features_dc):
    """f64 host math mirroring the reference's per-gaussian transforms."""
    means = np.tanh(xyz.astype(np.float64))
    cx = 0.5 * W * (means[..., 0] + 1.0)               # (T,N)
    cy = 0.5 * H * (means[..., 1] + 1.0)
    chol = cholesky.astype(np.float64) + np.array([0.5, 0.0, 0.5])
    l0, l1, l2 = chol[..., 0], chol[..., 1], chol[..., 2]
    sxx, sxy, syy = l0 * l0, l0 * l1, l1 * l1 + l2 * l2
    det = sxx * syy - sxy * sxy
    ca, cb_, cc = syy / det, -sxy / det, sxx / det     # conic
    tr = sxx + syy
    lam = tr / 2 + np.sqrt(np.maximum(tr * tr / 4 - det, 0.0))
    opac = 1.0 / (1.0 + np.exp(-opacity.astype(np.float64)[:, 0]))   # (N,)
    colors = 1.0 / (1.0 + np.exp(-features_dc.astype(np.float64)))   # (N,3)
    wcol = colors * opac[:, None]
    return cx, cy, ca, cb_, cc, lam, wcol


def _bin_entries(cx, cy, lam, S):
    """Route gaussians to 16x16 tiles; drop only provably-invisible overflow."""
    r = np.sqrt(2.0 * SIGMA_CUT * np.maximum(lam, 0.0)) + 1.0
    entries = []
    for t in range(T):
        x0 = np.clip(((cx[t] - r[t]) // TILE).astype(int), 0, NT - 1)
        x1 = np.clip(((cx[t] + r[t]) // TILE).astype(int), 0, NT - 1)
        y0 = np.clip(((cy[t] - r[t]) // TILE).astype(int), 0, NT - 1)
        y1 = np.clip(((cy[t] + r[t]) // TILE).astype(int), 0, NT - 1)
        buckets = [[[] for _ in range(NT)] for _ in range(NT)]
        for n in range(N):
            for ty in range(y0[n], y1[n] + 1):
                for tx in range(x0[n], x1[n] + 1):
                    buckets[ty][tx].append(n)
        for ty in range(NT):
            for tx in range(NT):
                # exact clipped-distance bound: sigma >= d^2/(2*lam_max)
                # everywhere on the tile, so pairs above DROP_MIN_SIGMA are
                # provably invisible (exp(-9) ~ 1e-4) - drop them
                idxs = []
                for n in buckets[ty][tx]:
                    ddx = max(0.0, tx * TILE - cx[t, n],
                              cx[t, n] - (tx * TILE + TILE - 1))
                    ddy = max(0.0, ty * TILE - cy[t, n],
                              cy[t, n] - (ty * TILE + TILE - 1))
                    if (ddx * ddx + ddy * ddy) / (2.0 * lam[t, n]) \
                            < DROP_MIN_SIGMA:
                        idxs.append(n)
                assert len(idxs) <= S, "tile overflow with visible gaussians"
                entries.append((t, ty, tx, idxs))
    return entries


def _pack(entries):
    """Global first-fit-decreasing pack of tiles onto 128-row groups."""
    sizes = np.array([len(e[3]) for e in entries])
    order = np.argsort(-sizes, kind="stable")
    bins = []            # list of [rows_used, [entry_idx, ...]]
    for i in order:
        s = int(sizes[i])
        if s == 0:
            continue
        for b in bins:
            if b[0] + s <= 128 and len(b[1]) < NE_CAP:
                b[0] += s
                b[1].append(int(i))
                break
        else:
            bins.append([s, [int(i)]])
    # deal bins (largest first) round-robin to cores
    core_bins = [[] for _ in range(N_CORES)]
    for j, b in enumerate(sorted(bins, key=lambda b: -b[0])):
        core_bins[j % N_CORES].append(b[1])
    G = max(len(cb) for cb in core_bins)
    # round up to a multiple of 32: engine APs need 32-aligned partition
    # bases; <=64 keeps two groups PE-placeable in one PSUM tile
    CHMAX = -(-3 * max(len(b[1]) for b in bins) // 32) * 32
    assert CHMAX <= 64, "NE_CAP too large for paired stage-2 placement"
    return core_bins, G, CHMAX


def _pack_core(bins, G, CHMAX, entries, cx, cy, ca, cb_, cc, wcol):
    """Build one core's coef ('cb' head), weight-block, and placement meta."""
    NCB = -(-G // 2)
    ct = np.zeros((24, NCB * 128), BF16)
    wb = np.zeros((128, G * CHMAX), BF16)
    meta = []
    for g, ent_list in enumerate(bins):
        b, gi = divmod(g, 2)
        r = 0
        for j, ei in enumerate(ent_list):
            t, ty, tx, idxs = entries[ei]
            n = np.asarray(idxs)
            k = len(n)
            ex = cx[t, n] - (tx * TILE + (TILE - 1) / 2.0)
            ey = cy[t, n] - (ty * TILE + (TILE - 1) / 2.0)
            A, Bc, Cc = ca[t, n], cb_[t, n], cc[t, n]
            C = np.empty((6, k), np.float64)
            C[0] = 0.5 * A
            C[1] = Bc
            C[2] = 0.5 * Cc
            C[3] = -(A * ex + Bc * ey)
            C[4] = -(Cc * ey + Bc * ex)
            C[5] = 0.5 * A * ex * ex + 0.5 * Cc * ey * ey + Bc * ex * ey
            hi = C.astype(BF16)
            lo = (C - hi.astype(np.float64)).astype(np.float32).astype(BF16)
            cols = slice(b * 128 + r, b * 128 + r + k)
            ct[12 * gi:12 * gi + 6, cols] = hi
            ct[12 * gi + 6:12 * gi + 12, cols] = lo
            wb[r:r + k, g * CHMAX + 3 * j:g * CHMAX + 3 * j + 3] = \
                wcol[n].astype(BF16)
            meta.append((g, j, ei))
            r += k
    return ct, wb, meta


def _basis_block():
    """[24, 2*256] bf16: centered half-integer monomials, exact in bf16."""
    x = np.arange(PIX, dtype=np.float64) % TILE - (TILE - 1) / 2.0
    y = np.arange(PIX, dtype=np.float64) // TILE - (TILE - 1) / 2.0
    rows = np.stack([x * x, x * y, y * y, x, y, np.ones(PIX)])  # (6,256)
    bt = np.zeros((24, 2 * PIX), np.float64)
    for gi in range(2):
        cols = slice(gi * PIX, (gi + 1) * PIX)
        bt[12 * gi:12 * gi + 6, cols] = rows
        bt[12 * gi + 6:12 * gi + 12, cols] = rows
    return bt.astype(BF16)


def _ensure_ntff_hook():
    """Provide antenv.axon_hooks (missing in this image) so trace=True works."""
    import sys, types, ctypes, contextlib
    if "antenv.axon_hooks" in sys.modules:
        return
    so_path = "/opt/axon/libaxon_pjrt.so"
    if not os.path.exists(so_path):
        return
    lib = ctypes.CDLL(so_path)
    if not hasattr(lib, "axon_start_nrt_profile"):
        return
    lib.axon_start_nrt_profile.argtypes = [ctypes.POINTER(ctypes.c_int64), ctypes.c_size_t]
    lib.axon_start_nrt_profile.restype = ctypes.c_int64
    lib.axon_stop_nrt_profile.argtypes = [ctypes.c_char_p]
    lib.axon_stop_nrt_profile.restype = ctypes.c_int64

    @contextlib.contextmanager
    def _hook(output_dir, device_ids):
        import jax
        jax.devices()
        if device_ids:
            ids = (ctypes.c_int64 * len(device_ids))(*device_ids)
            rc = lib.axon_start_nrt_profile(ids, len(device_ids))
        else:
            rc = lib.axon_start_nrt_profile(None, 0)
        if rc != 0:
            raise RuntimeError(f"axon_start_nrt_profile rc={rc}")
        try:
            yield
        finally:
            n = lib.axon_stop_nrt_profile(str(output_dir).encode())
            print(f"profile: {n} file(s) written to {output_dir}")

    mod = types.ModuleType("antenv.axon_hooks")
    mod.get_axon_ntff_profile_hook = lambda: _hook
    mod.set_axon_ntff_profile_hook = lambda h: None
    sys.modules["antenv.axon_hooks"] = mod


def kernel(xyz, cholesky, opacity, features_dc):
    from concourse import bass_utils

    xyz = np.asarray(xyz, np.float32)
    cholesky = np.asarray(cholesky, np.float32)
    opacity = np.asarray(opacity, np.float32)
    features_dc = np.asarray(features_dc, np.float32)

    cx, cy, ca, cb_, cc, lam, wcol = _host_params(
        xyz, cholesky, opacity, features_dc)

    entries = _bin_entries(cx, cy, lam, 128)
    core_bins, G, CHMAX = _pack(entries)

    if (G, CHMAX) not in _CACHE:
        _CACHE[(G, CHMAX)] = _build_nc(G, CHMAX)
    nc = _CACHE[(G, CHMAX)]

    bt = _basis_block()
    in_maps = []
    metas = []
    for c in range(N_CORES):
        ct, wb, meta = _pack_core(core_bins[c], G, CHMAX, entries,
                                  cx, cy, ca, cb_, cc, wcol)
        cbfull = np.concatenate([ct, bt], axis=1)
        half = cbfull.shape[1] // 2
        in_maps.append({"cb": np.concatenate(
                            [cbfull[:, 0:half], cbfull[:, half:]], axis=0),
                        "wblk": wb})
        metas.append(meta)

    trace = bool(int(os.environ.get("GS_TRACE", "0")))
    _ensure_ntff_hook()   # harmless if tracing is off; needed if forced on
    res = bass_utils.run_bass_kernel_spmd(
        nc, in_maps, core_ids=list(range(N_CORES)), trace=trace)
    kernel.last_result = res

    img = np.zeros((T, 3, H, W), np.float32)
    for c in range(N_CORES):
        o = res.results[c]["out"].astype(np.float32)
        for g, j, ei in metas[c]:
            t, ty, tx, _ = entries[ei]
            blk = o[(g % 2) * CHMAX + 3 * j:(g % 2) * CHMAX + 3 * j + 3,
                    (g // 2) * PIX:(g // 2 + 1) * PIX]
            img[t, :, ty * TILE:(ty + 1) * TILE,
                tx * TILE:(tx + 1) * TILE] = blk.reshape(3, TILE, TILE)
    return img
